# revision 1
# baseline (speedup 1.0000x reference)
"""Segment-softmax feature aggregation (segment_reduce) for Trainium2.

Full inputs: x [8, 256, 128, 128] f32, preds [8, 19, 128, 128] f32.
Sharded batch-parallel across 8 NeuronCores (1 batch per core).

Per-core algorithm (B=1, C=256, N=16384 pixels, K=19 classes):
  s[n]    = max_k preds[k, n]              (per-pixel max logit)
  mask    = (preds == s)                   one-hot argmax (input has no ties)
  w[n]    = exp(s[n])                      (s in [-0.25, 4.9] -> no max-sub needed)
  wm      = mask * w                       [n, k] layout
  aggT    = sum_n wm[n,:]^T (.) [xT[n,:] | 1]   PE matmul accumulation -> [k, C+1]
            (column C carries denom_k = sum_n w[n] mask[k,n])
  aggN    = aggT[:, :C] / denom            [k, C]
  out     = aggN^T @ maskB                 PE scatter matmul, maskB = mask^T [k, n]

All transposes of f32 data go through the PE (transpose mode w/ identity).
"""

import numpy as np

B, C, H, W, K = 8, 256, 128, 128, 19
N = H * W                  # 16384
TILE = 128                 # pixels per transpose tile
NT = N // TILE             # 128 n-tiles
XCH = 2048                 # x / out DMA chunk (1 MiB per 128-partition chunk)
NQ = N // XCH              # 8 chunks
PCH = 2048                 # preds DMA chunk
NCORES = 8

_CACHE = {}


def _build_nc():
    import concourse.bacc as bacc
    import concourse.tile as tile
    from concourse import mybir

    f32 = mybir.dt.float32
    Alu = mybir.AluOpType

    nc = bacc.Bacc("TRN2", target_bir_lowering=True)
    x_d = nc.dram_tensor("x", [C, N], f32, kind="ExternalInput")
    p_d = nc.dram_tensor("preds", [K, N], f32, kind="ExternalInput")
    e_d = nc.dram_tensor("ident", [128, 128], f32, kind="ExternalInput")
    o_d = nc.dram_tensor("out", [C, N], f32, kind="ExternalOutput")

    with tile.TileContext(nc) as tc:
        with tc.tile_pool(name="singles", bufs=1) as singles:
            ident = singles.tile([128, 128], f32)
            nc.sync.dma_start(out=ident, in_=e_d[:])
            s_all = singles.tile([128, NT], f32)
            w_all = singles.tile([128, NT], f32)
            maskA = singles.tile([128, NT, K], f32)
            maskB = singles.tile([K, N], f32)
            aggT = singles.tile([K, C + 1], f32)
            aggNT = singles.tile([K, C], f32)

            # ---- Phase 1: preds -> s, maskA --------------------------------
            with (
                tc.tile_pool(name="pch", bufs=2) as pchp,
                tc.tile_pool(name="psA", bufs=2, space="PSUM") as psAp,
                tc.tile_pool(name="pA", bufs=3) as pAp,
            ):
                for q in range(N // PCH):
                    pch = pchp.tile([K, PCH], f32)
                    nc.sync.dma_start(out=pch, in_=p_d[:, q * PCH:(q + 1) * PCH])
                    for g in range(PCH // (4 * TILE)):          # groups of 4 tiles
                        psA = psAp.tile([128, 4 * K], f32)
                        for j in range(4):
                            off = (g * 4 + j) * TILE
                            nc.tensor.transpose(
                                psA[:, j * K:(j + 1) * K],
                                pch[:, off:off + TILE],
                                ident[0:K, 0:K],
                            )
                        pa = pAp.tile([128, 4 * K], f32)
                        nc.vector.tensor_copy(pa, psA)
                        i0 = q * (PCH // TILE) + g * 4
                        nc.vector.tensor_reduce(
                            s_all[:, i0:i0 + 4],
                            pa.rearrange("p (t k) -> p t k", k=K),
                            axis=mybir.AxisListType.X,
                            op=Alu.max,
                        )
                        for j in range(4):
                            i = i0 + j
                            nc.vector.tensor_scalar(
                                maskA[:, i, :],
                                pa[:, j * K:(j + 1) * K],
                                s_all[:, i:i + 1],
                                None,
                                Alu.is_equal,
                            )

            # ---- Phase 1.5: w = exp(s) -------------------------------------
            nc.scalar.activation(w_all, s_all, mybir.ActivationFunctionType.Exp)

            # ---- Phase 2: xT tiles, agg accumulation, maskB ----------------
            with (
                tc.tile_pool(name="xch", bufs=2) as xchp,
                tc.tile_pool(name="xT", bufs=3) as xTp,
                tc.tile_pool(name="wm", bufs=3) as wmp,
                tc.tile_pool(name="psXT", bufs=3, space="PSUM") as psXTp,
                tc.tile_pool(name="psMB", bufs=2, space="PSUM") as psMBp,
                tc.tile_pool(name="psAgg", bufs=1, space="PSUM") as psAggp,
            ):
                psAgg = psAggp.tile([K, C + 1], f32)
                for q in range(NQ):
                    xc0 = xchp.tile([128, XCH], f32, tag="xch0")
                    xc1 = xchp.tile([128, XCH], f32, tag="xch1")
                    nc.sync.dma_start(out=xc0, in_=x_d[0:128, q * XCH:(q + 1) * XCH])
                    nc.sync.dma_start(out=xc1, in_=x_d[128:256, q * XCH:(q + 1) * XCH])
                    for g in range(XCH // (4 * TILE)):          # 4 groups of 4 tiles
                        psMB = psMBp.tile([K, 4 * TILE], f32)
                        for j in range(4):
                            sub = g * 4 + j                     # sub-tile in chunk
                            i = q * (XCH // TILE) + sub         # global n-tile
                            psXT = psXTp.tile([128, 256], f32)
                            nc.tensor.transpose(
                                psXT[:, 0:128],
                                xc0[:, sub * TILE:(sub + 1) * TILE],
                                ident,
                            )
                            nc.tensor.transpose(
                                psXT[:, 128:256],
                                xc1[:, sub * TILE:(sub + 1) * TILE],
                                ident,
                            )
                            xT = xTp.tile([128, C + 1], f32)
                            nc.scalar.copy(xT[:, 0:C], psXT)
                            nc.gpsimd.memset(xT[:, C:C + 1], 1.0)
                            wm = wmp.tile([128, K], f32)
                            nc.vector.tensor_scalar(
                                wm, maskA[:, i, :], w_all[:, i:i + 1], None, Alu.mult
                            )
                            nc.tensor.matmul(
                                psAgg, lhsT=wm, rhs=xT,
                                start=(i == 0), stop=(i == NT - 1),
                            )
                            nc.tensor.transpose(
                                psMB[:, j * TILE:(j + 1) * TILE],
                                maskA[:, i, :],
                                ident,
                            )
                        nb0 = (q * (XCH // TILE) + g * 4) * TILE
                        nc.vector.tensor_copy(maskB[:, nb0:nb0 + 4 * TILE], psMB)

                # ---- Phase 3: normalize ------------------------------------
                nc.vector.tensor_copy(aggT, psAgg)
                nc.vector.tensor_scalar(
                    aggT[:, C:C + 1], aggT[:, C:C + 1], 1e-30, None, Alu.max
                )
                dinv = singles.tile([K, 1], f32)
                nc.vector.reciprocal(dinv, aggT[:, C:C + 1])
                nc.vector.tensor_scalar(
                    aggNT, aggT[:, 0:C], dinv, None, Alu.mult
                )

            # ---- Phase 4: scatter out = aggN^T @ maskB ---------------------
            with (
                tc.tile_pool(name="psO", bufs=3, space="PSUM") as psOp,
                tc.tile_pool(name="ost", bufs=3) as ostp,
            ):
                for h in range(2):
                    for q in range(NQ):
                        ost = ostp.tile([128, XCH], f32)
                        for j in range(XCH // 512):
                            psO = psOp.tile([128, 512], f32)
                            nb0 = q * XCH + j * 512
                            nc.tensor.matmul(
                                psO,
                                lhsT=aggNT[:, h * 128:(h + 1) * 128],
                                rhs=maskB[:, nb0:nb0 + 512],
                                start=True, stop=True,
                            )
                            if j % 2 == 0:
                                nc.vector.tensor_copy(ost[:, j * 512:(j + 1) * 512], psO)
                            else:
                                nc.scalar.copy(ost[:, j * 512:(j + 1) * 512], psO)
                        nc.scalar.dma_start(
                            out=o_d[h * 128:(h + 1) * 128, q * XCH:(q + 1) * XCH],
                            in_=ost,
                        )

    nc.compile()
    return nc


def _get_nc():
    if "nc" not in _CACHE:
        _CACHE["nc"] = _build_nc()
    return _CACHE["nc"]


def kernel(x, preds):
    from concourse.bass_utils import run_bass_kernel_spmd

    x = np.asarray(x, dtype=np.float32)
    preds = np.asarray(preds, dtype=np.float32)
    ident = np.eye(128, dtype=np.float32)

    nc = _get_nc()
    in_maps = [
        {
            "x": np.ascontiguousarray(x[b].reshape(C, N)),
            "preds": np.ascontiguousarray(preds[b].reshape(K, N)),
            "ident": ident,
        }
        for b in range(NCORES)
    ]
    res = run_bass_kernel_spmd(nc, in_maps, list(range(NCORES)))
    out = np.stack(
        [np.asarray(res.results[b]["out"]).reshape(C, H, W) for b in range(NCORES)]
    )
    return out



# revision 19
# speedup vs baseline: 1.0565x; 1.0565x over previous
"""Segment-softmax feature aggregation (segment_reduce) for Trainium2.

Full inputs: x [8, 256, 128, 128] f32, preds [8, 19, 128, 128] f32.
Sharded batch-parallel across 8 NeuronCores (1 batch per core).

Per-core algorithm (B=1, C=256, N=16384 pixels, K=19 classes):
  s[n]   = max_k preds[k, n]                (per-pixel max logit)
  mask   = (preds == s)                     one-hot argmax (no ties in input)
  wm     = exp(preds) * mask = exp(s)*mask  [n, k] tiles (lhsT)
  agg    = sum_n wm[n,:]^T (.) xT[n,:]      PE accumulation -> [k, C]
  den    = sum_n wm[n,:]^T (.) 1            PE accumulation -> [k, 1]
  aggN   = agg / max(den, 1e-30)            (cast bf16)
  out    = aggN^T @ mask[k, n]              PE scatter matmul (bf16)

Transposes run fp32 on the PE; all true matmuls run bf16 (1 cyc/col vs
4 for fp32).  preds is loaded in a packed 32-padded layout [4*32, 4096]
(4 pixel-tiles j on partition blocks j*32+k, pixel n = (4a+j)*128+p at
free index a*128+p) so ONE PE transpose yields 4 tiles of [pixel, k],
and the packed copy doubles as the phase-4 mask source: mask is
computed in the packed layout by comparing against s broadcast through
a tiny HBM round-trip (s^T stored pixel-linear, re-read replicated).
"""

import numpy as np

B, C, H, W, K = 8, 256, 128, 128, 19
N = H * W                  # 16384
TILE = 128                 # pixels per transpose tile
NT = N // TILE             # 128 n-tiles
NG = NT // 4               # 32 groups of 4 tiles
QF = NG * TILE             # 4096 packed free size
XCH = 2048                 # x / out chunk (pixels)
NQ = N // XCH              # 8 chunks
NCORES = 8

_CACHE = {}


def _build_nc():
    import concourse.bacc as bacc
    import concourse.tile as tile
    from concourse import mybir

    f32 = mybir.dt.float32
    bf16 = mybir.dt.bfloat16
    Alu = mybir.AluOpType
    Act = mybir.ActivationFunctionType

    nc = bacc.Bacc("TRN2", target_bir_lowering=True)
    x_d = nc.dram_tensor("x", [C, N], f32, kind="ExternalInput")
    p_d = nc.dram_tensor("preds", [K, N], f32, kind="ExternalInput")
    e_d = nc.dram_tensor("ident", [128, 128], f32, kind="ExternalInput")
    o_d = nc.dram_tensor("out", [C, N], f32, kind="ExternalOutput")
    srow_d = nc.dram_tensor("srow", [1, N], f32, kind="Internal")

    # packed preds view: [j, k, a, p] with n = (4a + j)*128 + p
    pq_src = p_d.rearrange("k (a j p) -> j k a p", j=4, p=TILE)

    with tile.TileContext(nc) as tc:
        with tc.tile_pool(name="singles", bufs=1) as singles:
            ident = singles.tile([128, 128], f32)
            nc.sync.dma_start(out=ident, in_=e_d[:])

            predsQ = singles.tile([128, QF], f32)   # j-blocks at j*32
            for j in range(4):
                nc.sync.dma_start(
                    out=predsQ[j * 32:j * 32 + K, :]
                    .rearrange("k (a p) -> k a p", p=TILE),
                    in_=pq_src[j],
                )

            s_all = singles.tile([128, NT], f32)
            sT = singles.tile([128, 128], f32)
            wmA = singles.tile([128, NT, K], bf16)
            s_repQ = singles.tile([128, QF], f32)
            maskQ = singles.tile([128, QF], bf16)
            maskQ3 = singles.tile([K, QF], bf16)   # j=3 (PE can't read p96+)
            aggNb = singles.tile([128, C], bf16)   # replicated at j*32
            dclamp = singles.tile([K, 1], f32)
            dinv = singles.tile([K, 1], f32)
            ones_col = singles.tile([128, 1], bf16)
            nc.vector.memset(ones_col, 1.0)
            xts = [
                singles.tile([128, C], bf16, name=f"xt{v}") for v in range(3)
            ]

            with (
                tc.tile_pool(name="mtmp", bufs=3) as mtp,
                tc.tile_pool(name="xch", bufs=2) as xchp,
                tc.tile_pool(name="psA", bufs=2, space="PSUM") as psAp,
                tc.tile_pool(name="psS", bufs=1, space="PSUM") as psSp,
                tc.tile_pool(name="psXT", bufs=3, space="PSUM") as psXTp,
                tc.tile_pool(name="psAgg", bufs=1, space="PSUM") as psAggp,
                tc.tile_pool(name="psDen", bufs=1, space="PSUM") as psDenp,
            ):
                psAgg = psAggp.tile([K, C], f32)
                psDen = psDenp.tile([K, 1], f32)

                # ---- Phase 1: preds -> s_all, wmA ---------------------------
                for g in range(NG):
                    psA = psAp.tile([128, 128], f32)
                    nc.tensor.transpose(
                        psA,
                        predsQ[:, g * TILE:(g + 1) * TILE],
                        ident,
                    )
                    psA3 = psA.rearrange("p (j w) -> p j w", w=32)[:, :, 0:K]
                    # wm = exp(preds) (ACT, dep on PE only)
                    nc.scalar.activation(
                        wmA[:, 4 * g:4 * g + 4, :], psA3, Act.Exp
                    )
                    # s = max_k
                    nc.vector.tensor_reduce(
                        s_all[:, 4 * g:4 * g + 4],
                        psA3,
                        axis=mybir.AxisListType.X,
                        op=Alu.max,
                    )
                    # mask = (preds == s); wm *= mask
                    mt = mtp.tile([128, 4, K], f32)
                    for j in range(4):
                        i = 4 * g + j
                        nc.vector.tensor_scalar(
                            mt[:, j, :],
                            psA[:, j * 32:j * 32 + K],
                            s_all[:, i:i + 1],
                            None,
                            Alu.is_equal,
                        )
                    nc.vector.tensor_tensor(
                        out=wmA[:, 4 * g:4 * g + 4, :],
                        in0=wmA[:, 4 * g:4 * g + 4, :],
                        in1=mt,
                        op=Alu.mult,
                    )

                # ---- Phase 2: x transpose + agg/den accumulation ------------
                xv = x_d.rearrange("(h p) n -> p h n", p=128)
                mm_queue = []

                def emit_mms():
                    xT, i = mm_queue.pop(0)
                    lhsT = wmA[:, i, :]
                    nc.tensor.matmul(
                        psAgg, lhsT=lhsT, rhs=xT,
                        start=(i == 0), stop=(i == NT - 1),
                    )
                    nc.tensor.matmul(
                        psDen, lhsT=lhsT, rhs=ones_col,
                        start=(i == 0), stop=(i == NT - 1),
                    )

                for q in range(NQ):
                    xch = xchp.tile([128, 2, XCH], f32)
                    nc.sync.dma_start(
                        out=xch, in_=xv[:, :, q * XCH:(q + 1) * XCH]
                    )
                    for sub in range(XCH // TILE):
                        i = q * (XCH // TILE) + sub
                        psXT = psXTp.tile([128, C], f32)
                        nc.tensor.transpose(
                            psXT[:, 0:128],
                            xch[:, 0, sub * TILE:(sub + 1) * TILE],
                            ident,
                        )
                        nc.tensor.transpose(
                            psXT[:, 128:256],
                            xch[:, 1, sub * TILE:(sub + 1) * TILE],
                            ident,
                        )
                        xT = xts[i % 3]
                        nc.scalar.copy(xT, psXT)
                        mm_queue.append((xT, i))
                        if len(mm_queue) > 1:
                            emit_mms()

                    # mid-kernel: s broadcast machinery (placed so PE/DVE
                    # reach it only after s_all is complete)
                    if q == 2:
                        psS = psSp.tile([128, 128], f32)
                        nc.tensor.transpose(psS, s_all, ident)
                        nc.vector.tensor_copy(sT, psS)
                        nc.gpsimd.dma_start(out=srow_d[:], in_=sT)
                        srow_j = srow_d.rearrange(
                            "one (a j p) -> (one j) a p", j=4, p=TILE
                        )
                        for j in range(4):
                            nc.gpsimd.dma_start(
                                out=s_repQ[j * 32:j * 32 + K, :]
                                .rearrange("k (a p) -> k a p", p=TILE),
                                in_=srow_j[j:j + 1]
                                .broadcast_to([K, NG, TILE]),
                            )
                    if q == 3:
                        nc.vector.tensor_tensor(
                            out=maskQ[0:83, :], in0=predsQ[0:83, :],
                            in1=s_repQ[0:83, :], op=Alu.is_equal,
                        )
                        nc.vector.tensor_tensor(
                            out=maskQ3, in0=predsQ[96:96 + K, :],
                            in1=s_repQ[96:96 + K, :], op=Alu.is_equal,
                        )

                while mm_queue:
                    emit_mms()

                # ---- Phase 3: normalize ------------------------------------
                nc.vector.tensor_scalar(
                    dclamp, psDen, 1e-30, None, Alu.max
                )
                nc.vector.reciprocal(dinv, dclamp)
                for j in range(3):
                    nc.vector.tensor_scalar(
                        aggNb[j * 32:j * 32 + K, :], psAgg, dinv, None,
                        Alu.mult,
                    )

            # ---- Phase 4: scatter out = aggN^T @ mask ----------------------
            with (
                tc.tile_pool(name="psO", bufs=4, space="PSUM") as psOp,
                tc.tile_pool(name="ost", bufs=4) as ostp,
            ):
                for q in range(NQ):
                    for h in range(2):
                        ost = ostp.tile([128, XCH], f32)
                        ostv = ost.rearrange(
                            "c (a j p) -> c a j p", j=4, p=TILE
                        )
                        for j in range(4):
                            psO = psOp.tile([128, 4 * TILE], f32)
                            jb = 0 if j == 3 else j * 32
                            rhs = (
                                maskQ3[:, q * 512:(q + 1) * 512]
                                if j == 3
                                else maskQ[jb:jb + K, q * 512:(q + 1) * 512]
                            )
                            nc.tensor.matmul(
                                psO,
                                lhsT=aggNb[jb:jb + K,
                                           h * 128:(h + 1) * 128],
                                rhs=rhs,
                                start=True, stop=True,
                            )
                            src = psO.rearrange("c (a p) -> c a p", p=TILE)
                            if j < 3:
                                nc.vector.tensor_copy(ostv[:, :, j, :], src)
                            else:
                                nc.scalar.copy(ostv[:, :, j, :], src)
                        if h == 0:
                            nc.sync.dma_start(
                                out=o_d[0:128, q * XCH:(q + 1) * XCH], in_=ost
                            )
                        else:
                            nc.scalar.dma_start(
                                out=o_d[128:256, q * XCH:(q + 1) * XCH],
                                in_=ost,
                            )

    nc.compile()
    return nc


def _get_nc():
    if "nc" not in _CACHE:
        _CACHE["nc"] = _build_nc()
    return _CACHE["nc"]


def kernel(x, preds):
    from concourse.bass_utils import run_bass_kernel_spmd

    x = np.asarray(x, dtype=np.float32)
    preds = np.asarray(preds, dtype=np.float32)
    ident = np.eye(128, dtype=np.float32)

    nc = _get_nc()
    in_maps = [
        {
            "x": np.ascontiguousarray(x[b].reshape(C, N)),
            "preds": np.ascontiguousarray(preds[b].reshape(K, N)),
            "ident": ident,
        }
        for b in range(NCORES)
    ]
    res = run_bass_kernel_spmd(nc, in_maps, list(range(NCORES)))
    out = np.stack(
        [np.asarray(res.results[b]["out"]).reshape(C, H, W) for b in range(NCORES)]
    )
    return out


# revision 23
# speedup vs baseline: 1.3680x; 1.2949x over previous
"""Segment-softmax feature aggregation (segment_reduce) for Trainium2.

Full inputs: x [8, 256, 128, 128] f32, preds [8, 19, 128, 128] f32.
Sharded batch-parallel across 8 NeuronCores (1 batch per core).

Per-core algorithm (B=1, C=256, N=16384 pixels, K=19 classes):
  s[n]   = max_k preds[k, n]                (per-pixel max logit)
  mask   = (preds == s)                     one-hot argmax (no ties in input)
  wm     = exp(preds) * mask = exp(s)*mask
  agg    = sum_n wm[n,:]^T (.) xT[n,:]      PE accumulation -> [k, C]
  den    = sum_n wm[n,:]^T (.) 1            PE accumulation -> [k, 1]
  aggN   = agg / max(den, 1e-30)            (cast bf16)
  out    = aggN^T @ mask[k, n]              PE scatter matmul (bf16)

Layout: preds/mask/wm live in a "packed" [128, 4096] layout: partition
j*32+k (j = chunk%4, k = class; 32-padding because PE operands must
start at partition 0/32/64), free (a, r) with a = chunk//4, r = pixel
within chunk (chunk = 2048 pixels).  Pixel n = (a*4+j)*2048 + r.  This
keeps every DMA descriptor an 8 KiB contiguous block and lets one PE
transpose of a [128, 128] packed slice produce 4 tiles of [pixel, k].
s is broadcast across classes via a tiny HBM round-trip (s^T stored
pixel-linear, re-read replicated per class).  All true matmuls run
bf16 (1 cyc/col); x transposes are fp32 on the PE.  Input chunks
alternate between the two HWDGE rings (sync/scalar) to reach full HBM
read bandwidth; output writes do the same.
"""

import numpy as np

B, C, H, W, K = 8, 256, 128, 128, 19
N = H * W                  # 16384
TILE = 128                 # pixels per transpose tile
NT = N // TILE             # 128 n-tiles
NG = NT // 4               # 32 packed groups
QF = NG * TILE             # 4096 packed free size
XCH = 2048                 # x / out chunk (pixels)
NQ = N // XCH              # 8 chunks
NCORES = 8

_CACHE = {}


def _build_nc():
    import concourse.bacc as bacc
    import concourse.tile as tile
    from concourse import mybir

    f32 = mybir.dt.float32
    bf16 = mybir.dt.bfloat16
    Alu = mybir.AluOpType
    Act = mybir.ActivationFunctionType

    nc = bacc.Bacc("TRN2", target_bir_lowering=True)
    x_d = nc.dram_tensor("x", [C, N], f32, kind="ExternalInput")
    p_d = nc.dram_tensor("preds", [K, N], f32, kind="ExternalInput")
    e_d = nc.dram_tensor("ident", [128, 128], f32, kind="ExternalInput")
    o_d = nc.dram_tensor("out", [C, N], f32, kind="ExternalOutput")
    srow_d = nc.dram_tensor("srow", [1, N], f32, kind="Internal")

    # packed views: [j, k, a, r] with n = (a*4 + j)*2048 + r
    pq_src = p_d.rearrange("k (a j r) -> j k a r", j=4, r=XCH)
    sq_src = srow_d.rearrange("one (a j r) -> j one a r", j=4, r=XCH)

    with tile.TileContext(nc) as tc:
        with tc.tile_pool(name="singles", bufs=1) as singles:
            ident = singles.tile([128, 128], f32)
            nc.sync.dma_start(out=ident, in_=e_d[:])
            identB = singles.tile([128, 128], bf16)
            nc.vector.tensor_copy(identB, ident)

            predsQ = singles.tile([128, QF], f32)   # j-blocks at j*32
            nc.gpsimd.memset(predsQ, 0.0)           # keep pad rows finite
            for j in range(4):
                eng = nc.sync if j < 2 else nc.scalar
                eng.dma_start(
                    out=predsQ[j * 32:j * 32 + K, :]
                    .rearrange("k (a r) -> k a r", r=XCH),
                    in_=pq_src[j],
                )

            s_all = singles.tile([128, NT], f32)
            sT = singles.tile([128, 128], f32)
            wmA = singles.tile([128, NT, K], bf16)
            wmQ = singles.tile([128, QF], bf16)
            s_repQ = singles.tile([128, QF], f32)
            maskQ = singles.tile([128, QF], bf16)
            maskQ3 = singles.tile([K, QF], bf16)   # j=3 (PE can't read p96+)
            aggNb = singles.tile([128, C], bf16)   # replicated at 0/32/64
            dclamp = singles.tile([K, 1], f32)
            dinv = singles.tile([K, 1], f32)
            # persistent transposed-x pair buffers: [n, 2, C+1] bf16
            # pair p covers tiles (2p, 2p+1); col C of each holds 1.0 so the
            # agg matmul's column C accumulates the softmax denominator
            xta = [
                singles.tile([128, 2, C + 1], bf16, name=f"xt{v}")
                for v in range(NT // 2)
            ]
            for v in range(NT // 2):
                nc.gpsimd.memset(xta[v][:, :, C:C + 1], 1.0)

            s_view = s_all.rearrange("p (a j t) -> p a j t", a=2, j=4)
            wm_view = wmA.rearrange("p (a j t) k -> p a j t k", a=2, j=4)

            with (
                tc.tile_pool(name="xch", bufs=2) as xchp,
                tc.tile_pool(name="psA", bufs=2, space="PSUM") as psAp,
                tc.tile_pool(name="psXT", bufs=3, space="PSUM") as psXTp,
                tc.tile_pool(name="psAgg", bufs=1, space="PSUM") as psAggp,
            ):
                psAgg = psAggp.tile([K, C + 1], f32)

                # ---- Phase 1: packed preds -> s_all -------------------------
                for g in range(NG):
                    psA = psAp.tile([128, 128], f32, name="psA")
                    nc.tensor.transpose(
                        psA, predsQ[:, g * TILE:(g + 1) * TILE], ident
                    )
                    psA3 = psA.rearrange("p (j w) -> p j w", w=32)[:, :, 0:K]
                    nc.vector.tensor_reduce(
                        s_view[:, g // 16, :, g % 16],
                        psA3,
                        axis=mybir.AxisListType.X,
                        op=Alu.max,
                    )

                evac_cnt = [0]

                def emit_chunk(c):
                    xch = xchp.tile([128, 2, XCH], f32, name="xch")
                    e0 = nc.sync if c % 2 == 0 else nc.scalar
                    e1 = nc.scalar if c % 2 == 0 else nc.sync
                    e0.dma_start(
                        out=xch[:, 0, :], in_=x_d[0:128, c * XCH:(c + 1) * XCH]
                    )
                    e1.dma_start(
                        out=xch[:, 1, :],
                        in_=x_d[128:256, c * XCH:(c + 1) * XCH],
                    )
                    for pp in range(XCH // (2 * TILE)):     # 8 pairs
                        pg = c * 8 + pp                     # global pair
                        psXT = psXTp.tile([128, 4 * TILE], f32, name="psXT")
                        for v in range(4):                  # (tile, half)
                            nc.tensor.transpose(
                                psXT[:, v * 128:(v + 1) * 128],
                                xch[:, v % 2, (2 * pp + v // 2) * TILE:
                                    (2 * pp + v // 2 + 1) * TILE],
                                ident,
                            )
                        eng = nc.vector if evac_cnt[0] % 2 == 0 else nc.scalar
                        evac_cnt[0] += 1
                        dst = xta[pg][:, :, 0:C]
                        if eng is nc.vector:
                            nc.vector.tensor_copy(dst, psXT)
                        else:
                            nc.scalar.copy(dst, psXT)

                def emit_mms(chunks):
                    for c in chunks:
                        for sub in range(XCH // TILE):
                            i = c * 16 + sub
                            nc.tensor.matmul(
                                psAgg, lhsT=wmA[:, i, :],
                                rhs=xta[i // 2][:, i % 2, :],
                                start=(i == 0), stop=(i == NT - 1),
                            )

                emit_chunk(0)
                emit_chunk(1)

                # s broadcast machinery (PE hits psS after c0/c1 transposes)
                psS = psAp.tile([128, 128], f32, name="psA")
                nc.tensor.transpose(psS, s_all, ident)
                nc.vector.tensor_copy(sT, psS)
                nc.gpsimd.dma_start(out=srow_d[:], in_=sT)
                for j in range(4):
                    nc.gpsimd.dma_start(
                        out=s_repQ[j * 32:j * 32 + K, :]
                        .rearrange("k (a r) -> k a r", r=XCH),
                        in_=sq_src[j].broadcast_to([K, 2, XCH]),
                    )

                emit_chunk(2)
                emit_chunk(3)

                # wm = exp(preds) * (preds == s), packed; then -> wmA tiles
                nc.scalar.activation(wmQ, predsQ, Act.Exp)
                nc.vector.tensor_tensor(
                    out=maskQ[0:115, :], in0=predsQ[0:115, :],
                    in1=s_repQ[0:115, :], op=Alu.is_equal,
                )
                nc.vector.tensor_tensor(
                    out=maskQ3, in0=predsQ[96:96 + K, :],
                    in1=s_repQ[96:96 + K, :], op=Alu.is_equal,
                )
                nc.vector.tensor_tensor(
                    out=wmQ[0:115, :], in0=wmQ[0:115, :],
                    in1=maskQ[0:115, :], op=Alu.mult,
                )

                emit_chunk(4)
                emit_chunk(5)

                for g in range(NG):
                    psWM = psAp.tile([128, 128], bf16, name="psWM")
                    nc.tensor.transpose(
                        psWM, wmQ[:, g * TILE:(g + 1) * TILE], identB
                    )
                    dst = wm_view[:, g // 16, :, g % 16, :]
                    src = psWM.rearrange("p (j w) -> p j w", w=32)[:, :, 0:K]
                    if g % 2 == 0:
                        nc.vector.tensor_copy(dst, src)
                    else:
                        nc.scalar.copy(dst, src)

                emit_mms(range(0, 4))
                emit_chunk(6)
                emit_chunk(7)
                emit_mms(range(4, 8))

                # ---- Phase 3: normalize ------------------------------------
                nc.vector.tensor_scalar(
                    dclamp, psAgg[:, C:C + 1], 1e-30, None, Alu.max
                )
                nc.vector.reciprocal(dinv, dclamp)
                for j in range(3):
                    nc.vector.tensor_scalar(
                        aggNb[j * 32:j * 32 + K, :], psAgg[:, 0:C], dinv,
                        None, Alu.mult,
                    )

            # ---- Phase 4: scatter out = aggN^T @ mask ----------------------
            with (
                tc.tile_pool(name="psO", bufs=4, space="PSUM") as psOp,
                tc.tile_pool(name="ost", bufs=4) as ostp,
            ):
                for q in range(NQ):
                    j, a = q % 4, q // 4
                    jb = 0 if j == 3 else j * 32
                    for h in range(2):
                        ost = ostp.tile([128, XCH], f32, name="ost")
                        for m in range(4):
                            psO = psOp.tile([128, 4 * TILE], f32, name="psO")
                            fs = a * XCH + m * 512
                            rhs = (
                                maskQ3[:, fs:fs + 512] if j == 3
                                else maskQ[jb:jb + K, fs:fs + 512]
                            )
                            nc.tensor.matmul(
                                psO,
                                lhsT=aggNb[jb:jb + K, h * 128:(h + 1) * 128],
                                rhs=rhs,
                                start=True, stop=True,
                            )
                            if m % 2 == 0:
                                nc.vector.tensor_copy(
                                    ost[:, m * 512:(m + 1) * 512], psO
                                )
                            else:
                                nc.scalar.copy(
                                    ost[:, m * 512:(m + 1) * 512], psO
                                )
                        eng = nc.sync if (q + h) % 2 == 0 else nc.scalar
                        eng.dma_start(
                            out=o_d[h * 128:(h + 1) * 128,
                                    q * XCH:(q + 1) * XCH],
                            in_=ost,
                        )

    nc.compile()
    return nc


def _get_nc():
    if "nc" not in _CACHE:
        _CACHE["nc"] = _build_nc()
    return _CACHE["nc"]


def kernel(x, preds):
    from concourse.bass_utils import run_bass_kernel_spmd

    x = np.asarray(x, dtype=np.float32)
    preds = np.asarray(preds, dtype=np.float32)
    ident = np.eye(128, dtype=np.float32)

    nc = _get_nc()
    in_maps = [
        {
            "x": np.ascontiguousarray(x[b].reshape(C, N)),
            "preds": np.ascontiguousarray(preds[b].reshape(K, N)),
            "ident": ident,
        }
        for b in range(NCORES)
    ]
    res = run_bass_kernel_spmd(nc, in_maps, list(range(NCORES)))
    out = np.stack(
        [np.asarray(res.results[b]["out"]).reshape(C, H, W) for b in range(NCORES)]
    )
    return out


# revision 24
# speedup vs baseline: 1.4221x; 1.0395x over previous
"""Segment-softmax feature aggregation (segment_reduce) for Trainium2.

Full inputs: x [8, 256, 128, 128] f32, preds [8, 19, 128, 128] f32.
Sharded batch-parallel across 8 NeuronCores (1 batch per core).

Per-core algorithm (B=1, C=256, N=16384 pixels, K=19 classes):
  s[n]   = max_k preds[k, n]                (per-pixel max logit)
  mask   = (preds == s)                     one-hot argmax (no ties in input)
  wm     = exp(preds) * mask = exp(s)*mask
  agg    = sum_n wm[n,:]^T (.) xT[n,:]      PE accumulation -> [k, C]
  den    = sum_n wm[n,:]^T (.) 1            PE accumulation -> [k, 1]
  aggN   = agg / max(den, 1e-30)            (cast bf16)
  out    = aggN^T @ mask[k, n]              PE scatter matmul (bf16)

Layout: preds/mask/wm live in a "packed" [128, 4096] layout: partition
j*32+k (j = chunk%4, k = class; 32-padding because PE operands must
start at partition 0/32/64), free (a, r) with a = chunk//4, r = pixel
within chunk (chunk = 2048 pixels).  Pixel n = (a*4+j)*2048 + r.  This
keeps every DMA descriptor an 8 KiB contiguous block and lets one PE
transpose of a [128, 128] packed slice produce 4 tiles of [pixel, k].
s is broadcast across classes via a tiny HBM round-trip (s^T stored
pixel-linear, re-read replicated per class).  All true matmuls run
bf16 (1 cyc/col); x transposes are fp32 on the PE.  Input chunks
alternate between the two HWDGE rings (sync/scalar) to reach full HBM
read bandwidth; output writes do the same.
"""

import numpy as np

B, C, H, W, K = 8, 256, 128, 128, 19
N = H * W                  # 16384
TILE = 128                 # pixels per transpose tile
NT = N // TILE             # 128 n-tiles
NG = NT // 4               # 32 packed groups
QF = NG * TILE             # 4096 packed free size
XCH = 2048                 # x / out chunk (pixels)
NQ = N // XCH              # 8 chunks
NCORES = 8

_CACHE = {}


def _build_nc():
    import concourse.bacc as bacc
    import concourse.tile as tile
    from concourse import mybir

    f32 = mybir.dt.float32
    bf16 = mybir.dt.bfloat16
    Alu = mybir.AluOpType
    Act = mybir.ActivationFunctionType

    nc = bacc.Bacc("TRN2", target_bir_lowering=True)
    x_d = nc.dram_tensor("x", [C, N], f32, kind="ExternalInput")
    p_d = nc.dram_tensor("preds", [K, N], f32, kind="ExternalInput")
    e_d = nc.dram_tensor("ident", [128, 128], f32, kind="ExternalInput")
    o_d = nc.dram_tensor("out", [C, N], f32, kind="ExternalOutput")
    srow_d = nc.dram_tensor("srow", [1, N], f32, kind="Internal")

    # packed views: [j, k, a, r] with n = (a*4 + j)*2048 + r
    pq_src = p_d.rearrange("k (a j r) -> j k a r", j=4, r=XCH)
    sq_src = srow_d.rearrange("one (a j r) -> j one a r", j=4, r=XCH)

    with tile.TileContext(nc) as tc:
        with tc.tile_pool(name="singles", bufs=1) as singles:
            ident = singles.tile([128, 128], f32)
            nc.sync.dma_start(out=ident, in_=e_d[:])
            identB = singles.tile([128, 128], bf16)
            nc.vector.tensor_copy(identB, ident)

            predsQ = singles.tile([128, QF], f32)   # j-blocks at j*32
            nc.vector.memset(predsQ, 0.0)           # keep pad rows finite
            for j in range(4):
                eng = nc.sync if j < 2 else nc.scalar
                eng.dma_start(
                    out=predsQ[j * 32:j * 32 + K, :]
                    .rearrange("k (a r) -> k a r", r=XCH),
                    in_=pq_src[j],
                )

            s_all = singles.tile([128, NT], f32)
            sT = singles.tile([128, 128], f32)
            wmA = singles.tile([128, NT, K], bf16)
            wmQ = singles.tile([128, QF], bf16)
            s_repQ = singles.tile([128, QF], f32)
            maskQ = singles.tile([128, QF], bf16)
            maskQ3 = singles.tile([K, QF], bf16)   # j=3 (PE can't read p96+)
            aggNb = singles.tile([128, C], bf16)   # replicated at 0/32/64
            dclamp = singles.tile([K, 1], f32)
            dinv = singles.tile([K, 1], f32)
            # persistent transposed-x buffer: [n, pair, tile, C+1] bf16;
            # col C holds 1.0 so the agg matmul's column C accumulates the
            # softmax denominator
            xtall = singles.tile([128, NT // 2, 2, C + 1], bf16)
            nc.gpsimd.memset(xtall[:, :, :, C:C + 1], 1.0)

            s_view = s_all.rearrange("p (a j t) -> p a j t", a=2, j=4)
            wm_view = wmA.rearrange("p (a j t) k -> p a j t k", a=2, j=4)

            with (
                tc.tile_pool(name="xch", bufs=3) as xchp,
                tc.tile_pool(name="psA", bufs=2, space="PSUM") as psAp,
                tc.tile_pool(name="psXT", bufs=3, space="PSUM") as psXTp,
                tc.tile_pool(name="psAgg", bufs=1, space="PSUM") as psAggp,
            ):
                psAgg = psAggp.tile([K, C + 1], f32)

                # ---- Phase 1: packed preds -> s_all -------------------------
                for g in range(NG):
                    psA = psAp.tile([128, 128], f32, name="psA")
                    nc.tensor.transpose(
                        psA, predsQ[:, g * TILE:(g + 1) * TILE], ident
                    )
                    psA3 = psA.rearrange("p (j w) -> p j w", w=32)[:, :, 0:K]
                    nc.vector.tensor_reduce(
                        s_view[:, g // 16, :, g % 16],
                        psA3,
                        axis=mybir.AxisListType.X,
                        op=Alu.max,
                    )

                evac_cnt = [0]

                def emit_chunk(c):
                    xch = xchp.tile([128, 2, XCH], f32, name="xch")
                    e0 = nc.sync if c % 2 == 0 else nc.scalar
                    e1 = nc.scalar if c % 2 == 0 else nc.sync
                    e0.dma_start(
                        out=xch[:, 0, :], in_=x_d[0:128, c * XCH:(c + 1) * XCH]
                    )
                    e1.dma_start(
                        out=xch[:, 1, :],
                        in_=x_d[128:256, c * XCH:(c + 1) * XCH],
                    )
                    for pp in range(XCH // (2 * TILE)):     # 8 pairs
                        pg = c * 8 + pp                     # global pair
                        psXT = psXTp.tile([128, 4 * TILE], f32, name="psXT")
                        for v in range(4):                  # (tile, half)
                            nc.tensor.transpose(
                                psXT[:, v * 128:(v + 1) * 128],
                                xch[:, v % 2, (2 * pp + v // 2) * TILE:
                                    (2 * pp + v // 2 + 1) * TILE],
                                ident,
                            )
                        eng = nc.vector if evac_cnt[0] % 2 == 0 else nc.scalar
                        evac_cnt[0] += 1
                        dst = xtall[:, pg, :, 0:C]
                        if eng is nc.vector:
                            nc.vector.tensor_copy(dst, psXT)
                        else:
                            nc.scalar.copy(dst, psXT)

                def emit_mms(chunks):
                    for c in chunks:
                        for sub in range(XCH // TILE):
                            i = c * 16 + sub
                            nc.tensor.matmul(
                                psAgg, lhsT=wmA[:, i, :],
                                rhs=xtall[:, i // 2, i % 2, :],
                                start=(i == 0), stop=(i == NT - 1),
                            )

                emit_chunk(0)
                emit_chunk(1)

                # s broadcast machinery (PE hits psS after c0/c1 transposes)
                psS = psAp.tile([128, 128], f32, name="psA")
                nc.tensor.transpose(psS, s_all, ident)
                nc.vector.tensor_copy(sT, psS)
                nc.gpsimd.dma_start(out=srow_d[:], in_=sT)
                for j in range(4):
                    nc.gpsimd.dma_start(
                        out=s_repQ[j * 32:j * 32 + K, :]
                        .rearrange("k (a r) -> k a r", r=XCH),
                        in_=sq_src[j].broadcast_to([K, 2, XCH]),
                    )

                emit_chunk(2)
                emit_chunk(3)

                # wm = exp(preds) * (preds == s), packed; then -> wmA tiles
                nc.scalar.activation(wmQ, predsQ, Act.Exp)
                nc.vector.tensor_tensor(
                    out=maskQ[0:115, :], in0=predsQ[0:115, :],
                    in1=s_repQ[0:115, :], op=Alu.is_equal,
                )
                nc.vector.tensor_tensor(
                    out=maskQ3, in0=predsQ[96:96 + K, :],
                    in1=s_repQ[96:96 + K, :], op=Alu.is_equal,
                )
                nc.vector.tensor_tensor(
                    out=wmQ[0:115, :], in0=wmQ[0:115, :],
                    in1=maskQ[0:115, :], op=Alu.mult,
                )

                emit_chunk(4)
                emit_chunk(5)

                for g in range(NG):
                    psWM = psAp.tile([128, 128], bf16, name="psWM")
                    nc.tensor.transpose(
                        psWM, wmQ[:, g * TILE:(g + 1) * TILE], identB
                    )
                    dst = wm_view[:, g // 16, :, g % 16, :]
                    src = psWM.rearrange("p (j w) -> p j w", w=32)[:, :, 0:K]
                    if g % 2 == 0:
                        nc.vector.tensor_copy(dst, src)
                    else:
                        nc.scalar.copy(dst, src)

                emit_mms(range(0, 4))
                emit_chunk(6)
                emit_chunk(7)
                emit_mms(range(4, 8))

                # ---- Phase 3: normalize ------------------------------------
                nc.vector.tensor_scalar(
                    dclamp, psAgg[:, C:C + 1], 1e-30, None, Alu.max
                )
                nc.vector.reciprocal(dinv, dclamp)
                for j in range(3):
                    nc.vector.tensor_scalar(
                        aggNb[j * 32:j * 32 + K, :], psAgg[:, 0:C], dinv,
                        None, Alu.mult,
                    )

            # ---- Phase 4: scatter out = aggN^T @ mask ----------------------
            with (
                tc.tile_pool(name="psO", bufs=4, space="PSUM") as psOp,
                tc.tile_pool(name="ost", bufs=3) as ostp,
            ):
                for q in range(NQ):
                    j, a = q % 4, q // 4
                    jb = 0 if j == 3 else j * 32
                    for h in range(2):
                        ost = ostp.tile([128, XCH], f32, name="ost")
                        for m in range(4):
                            psO = psOp.tile([128, 4 * TILE], f32, name="psO")
                            fs = a * XCH + m * 512
                            rhs = (
                                maskQ3[:, fs:fs + 512] if j == 3
                                else maskQ[jb:jb + K, fs:fs + 512]
                            )
                            nc.tensor.matmul(
                                psO,
                                lhsT=aggNb[jb:jb + K, h * 128:(h + 1) * 128],
                                rhs=rhs,
                                start=True, stop=True,
                            )
                            if m % 2 == 0:
                                nc.vector.tensor_copy(
                                    ost[:, m * 512:(m + 1) * 512], psO
                                )
                            else:
                                nc.scalar.copy(
                                    ost[:, m * 512:(m + 1) * 512], psO
                                )
                        eng = nc.sync if (q + h) % 2 == 0 else nc.scalar
                        eng.dma_start(
                            out=o_d[h * 128:(h + 1) * 128,
                                    q * XCH:(q + 1) * XCH],
                            in_=ost,
                        )

    nc.compile()
    return nc


def _get_nc():
    if "nc" not in _CACHE:
        _CACHE["nc"] = _build_nc()
    return _CACHE["nc"]


def kernel(x, preds):
    from concourse.bass_utils import run_bass_kernel_spmd

    x = np.asarray(x, dtype=np.float32)
    preds = np.asarray(preds, dtype=np.float32)
    ident = np.eye(128, dtype=np.float32)

    nc = _get_nc()
    in_maps = [
        {
            "x": np.ascontiguousarray(x[b].reshape(C, N)),
            "preds": np.ascontiguousarray(preds[b].reshape(K, N)),
            "ident": ident,
        }
        for b in range(NCORES)
    ]
    res = run_bass_kernel_spmd(nc, in_maps, list(range(NCORES)))
    out = np.stack(
        [np.asarray(res.results[b]["out"]).reshape(C, H, W) for b in range(NCORES)]
    )
    return out
